# revision 41
# baseline (speedup 1.0000x reference)
"""Block-diagonal (per-frame) multi-head attention on 8 Trainium2 cores.

Problem: x[2,3200,512] -> QKV proj (H=8 heads, D=64) -> attention masked to
25-token frames (128 frames) -> out[2,3200,512].  N = 3200 = 128*25.

Sharding: 256 (batch, frame) groups; core c handles batch c//4, frames
(c%4)*32..+32  => 800 tokens/core, tiled as 8 x 100 tokens (4 frames).

Layout trick: host sends x pre-transposed (xT [512, 800]) so every matmul
contracts over the partition dim:
  qT/kT [feat, tok] = W.T @ xT   (lhsT = W slice, rhs = xT)    [f32r MMs]
  v     [tok, feat] = xT.T @ Wv  (lhsT = xT slice, rhs = Wv)   [f32r MMs]
Scores per (head, tile): S^T = kT_h.T @ qT_h in bf16.  The -9e15 frame mask
is rank-5 (ones + 4 frame indicators), injected by one small matmul that
initializes the PSUM accumulation group.  softmax skips max-subtraction
(|scores| <~ 8).  PV uses E^T = exp(S^T) bf16 as stationary, v bf16 moving;
a ones-column in v makes PV's last column the softmax denominator.

Key optimizations vs the f32 baseline (150us -> ~67us):
  - proj matmuls f32r (1 cyc/row at N>=256), scores+PV bf16  (4x less PE)
  - v bias folded into the psum group as a rank-1 ones x bias matmul
  - stE/stO share one 2-bank psum tile -> single exp ACT per unit (3D AP)
  - normalization: 1 reciprocal + 1 broadcast tensor_mul per unit (was 4)
  - q evac on ACT, k evac on DVE (engine balance)
  - input DMAs alternate the two HWDGE rings (sync+scalar) in consumption
    order (wq_k, xt_k pairs first), so the first proj matmul starts ~11us
    instead of ~13us and the stream stays just behind the PE
  - bf16 stationaries widened to 128 columns (scores: kt slice, PV: et2
    slice) to trigger Fast Weight Load; junk columns only produce junk in
    output partitions 100..127, which nothing reads
  - last tile's output DMA is split per head-group to trim the kernel tail
"""

import numpy as np

B, N, DIN = 2, 3200, 512
H, D = 8, 64
TL, JN = 128, 25
NCORES = 8
TOK = 800      # tokens per core
NT = 8         # token tiles per core
TT = 100       # tokens per tile (4 frames)

# matmul dtype per stage: 'f32' | 'f32r' | 'bf16'
CONFIG = {"proj": "f32r", "qk": "bf16", "pv": "bf16"}

_CACHE = {}
LAST_RESULT = None  # BassKernelResults of the most recent kernel() call


def _build(cfg):
    import concourse.bacc as bacc
    import concourse.tile as tile
    from concourse import mybir

    f32 = mybir.dt.float32
    bf16 = mybir.dt.bfloat16
    f32r = mybir.dt.float32r
    AF = mybir.ActivationFunctionType

    def io_dt(kind):
        return {"f32": f32, "f32r": f32r, "bf16": bf16}[kind]

    proj_dt, qk_dt, pv_dt = cfg["proj"], cfg["qk"], cfg["pv"]

    nc = bacc.Bacc("TRN2", target_bir_lowering=False, debug=False,
                   num_devices=NCORES)

    xt_d = nc.dram_tensor("xT", [DIN, TOK], io_dt(proj_dt),
                          kind="ExternalInput").ap()
    w_d = {}
    for nm in ("wq", "wk", "wv"):
        w_d[nm] = nc.dram_tensor(nm, [DIN, DIN], io_dt(proj_dt),
                                 kind="ExternalInput").ap()
    bqc_d = nc.dram_tensor("bqc", [128, 4], f32, kind="ExternalInput").ap()
    bkc_d = nc.dram_tensor("bkc", [128, 4], f32, kind="ExternalInput").ap()
    bvr_d = nc.dram_tensor("bvr", [1, DIN], bf16, kind="ExternalInput").ap()
    ma_d = nc.dram_tensor("mA", [128, 128], bf16, kind="ExternalInput").ap()
    mb2_d = nc.dram_tensor("mB2", [128, 2 * TT], bf16,
                           kind="ExternalInput").ap()
    out_d = nc.dram_tensor("out", [TOK, DIN], f32, kind="ExternalOutput").ap()

    with tile.TileContext(nc) as tc:
        with (
            tc.tile_pool(name="persist", bufs=1) as pp,
            tc.tile_pool(name="scratch", bufs=4) as sp,
        ):
            # ---- persistent tiles ----
            wq = [pp.tile([128, DIN], io_dt(proj_dt), name=f"wq{k}",
                          tag=f"wq{k}") for k in range(4)]
            wk = [pp.tile([128, DIN], io_dt(proj_dt), name=f"wk{k}",
                          tag=f"wk{k}") for k in range(4)]
            wv = [pp.tile([128, DIN], io_dt(proj_dt), name=f"wv{k}",
                          tag=f"wv{k}") for k in range(4)]
            xt = [pp.tile([128, TOK], io_dt(proj_dt), name=f"xt{k}",
                          tag=f"xt{k}") for k in range(4)]
            bqc = pp.tile([128, 4], f32, name="bqc", tag="bqc")
            bkc = pp.tile([128, 4], f32, name="bkc", tag="bkc")
            bvr = pp.tile([1, DIN], bf16, name="bvr", tag="bvr")
            ones1 = pp.tile([1, TT], bf16, name="ones1", tag="ones1")
            zrow = pp.tile([1, DIN], bf16, name="zrow", tag="zrow")
            ma = pp.tile([128, 128], bf16, name="ma", tag="ma")
            mb2 = pp.tile([128, 2 * TT], bf16, name="mb2", tag="mb2")

            qt = [pp.tile([128, TOK], io_dt(qk_dt), name=f"qt{k}",
                          tag=f"qt{k}") for k in range(4)]
            # kt padded to 832 cols so 128-wide stationary slices stay in
            # bounds for the last tile (FWL needs 128-column weights)
            kt_ = [pp.tile([128, TOK + 32], io_dt(qk_dt), name=f"kt{k}",
                           tag=f"kt{k}") for k in range(4)]
            # v with 65 columns per head: col h*65+64 is all-ones so the PV
            # matmul also produces the softmax denominator in its last column
            vt = [pp.tile([TT, H * (D + 1)], io_dt(pv_dt), name=f"vt{t}",
                          tag=f"vt{t}") for t in range(NT)]
            ot = [pp.tile([TT, DIN], f32, name=f"ot{t}", tag=f"ot{t}")
                  for t in range(NT)]

            # constants with no DMA dependency
            nc.gpsimd.memset(ones1[:], 1.0)
            nc.gpsimd.memset(zrow[:], 0.0)
            for t in range(NT):
                vones = vt[t].rearrange("p (h c) -> p h c",
                                        c=D + 1)[:, :, D:D + 1]
                nc.gpsimd.memset(vones, 1.0)

            # ---- DMA in: two HWDGE rings (sync / scalar), consumption
            # order: bvr first (feeds the PE warm-up), then (wq_k, xt_k)
            # pairs, then wk, wv, biases, masks ----
            nc.scalar.dma_start(out=bvr, in_=bvr_d)
            for k in range(4):
                nc.sync.dma_start(out=wq[k], in_=w_d["wq"][k * 128:(k + 1) * 128, :])
                nc.scalar.dma_start(out=xt[k][:, 0:400],
                                    in_=xt_d[k * 128:(k + 1) * 128, 0:400])
                nc.sync.dma_start(out=xt[k][:, 400:800],
                                  in_=xt_d[k * 128:(k + 1) * 128, 400:800])
            nc.scalar.dma_start(out=bqc, in_=bqc_d)
            for k in range(4):
                (nc.sync if k % 2 else nc.scalar).dma_start(
                    out=wk[k], in_=w_d["wk"][k * 128:(k + 1) * 128, :])
            nc.scalar.dma_start(out=bkc, in_=bkc_d)
            for k in range(4):
                (nc.sync if k % 2 else nc.scalar).dma_start(
                    out=wv[k], in_=w_d["wv"][k * 128:(k + 1) * 128, :])
            nc.sync.dma_start(out=ma, in_=ma_d)
            nc.sync.dma_start(out=mb2, in_=mb2_d)

            with tc.tile_pool(name="vpsum", bufs=2, space="PSUM") as vps:
              with tc.tile_pool(name="ppsum", bufs=4, space="PSUM") as pps:
                # ---- HAM warm-up: rank-1 zero matmuls (numerically exact)
                # accumulated into the first two v psum groups.  They depend
                # only on memsets + the tiny bvr DMA, so the PE does ~4us of
                # dense work during the DMA-starved head and reaches full
                # clock (K=8/8) before the real projections start. ----
                vacc01 = []
                for t in range(2):
                    acc = vps.tile([TT, DIN], f32, name="vacc", tag="v",
                                   bufs=2)
                    vacc01.append(acc)
                    nc.tensor.matmul(acc[:], ones1[0:1, :], bvr[0:1, :],
                                     start=True, stop=False,
                                     skip_group_check=True)
                    for _ in range(5):
                        nc.tensor.matmul(acc[:], ones1[0:1, :], zrow[0:1, :],
                                         start=False, stop=False,
                                         skip_group_check=True)

                # ---- q^T / k^T projections: psum[feat, tok] ----
                # (ft, k, ch) order: the two ch matmuls share one stationary
                for (w, bc, dst, evac) in ((wq, bqc, qt, "act"),
                                           (wk, bkc, kt_, "dve")):
                    for ft in range(4):
                        fsl = slice(ft * 128, (ft + 1) * 128)
                        acc = [pps.tile([128, 400], f32, name=f"pacc{ch}",
                                        tag="p", bufs=4) for ch in range(2)]
                        for k in range(4):
                            for ch in range(2):
                                csl = slice(ch * 400, (ch + 1) * 400)
                                nc.tensor.matmul(
                                    acc[ch][:], w[k][:, fsl], xt[k][:, csl],
                                    start=(k == 0), stop=(k == 3))
                        for ch in range(2):
                            csl = slice(ch * 400, (ch + 1) * 400)
                            if evac == "act":
                                nc.scalar.activation(dst[ft][:, csl],
                                                     acc[ch][:], AF.Identity,
                                                     bias=bc[:, ft:ft + 1])
                            else:
                                nc.vector.tensor_scalar_add(
                                    dst[ft][:, csl], acc[ch][:],
                                    bc[:, ft:ft + 1])

              # ---- v projection interleaved with attention ----
              # v_t's dense f32r matmuls (N=512 streams) fill the PE bubbles
              # between the small bf16 attention matmuls, keeping the HAM
              # activity monitor above the re-throttle threshold through the
              # attention phase.  vpsum (2) + st2 (2x2) + pv4 (2) = 8 banks.
              # Per (tile, head-group-of-4) unit: one 2-bank psum tile st2
              # holds stE (cols 0:200, even heads, base-partition 0) and stO
              # (cols 512:712, odd heads, base-partition 64) so the PE's
              # row-group-concurrent matmuls never co-write a bank.  One exp
              # ACT covers both via a 3D AP.
              with tc.tile_pool(name="apsum", bufs=2, space="PSUM") as aps, \
                   tc.tile_pool(name="ppsum2", bufs=2, space="PSUM") as pps2:
                for t in range(NT):
                    tsl = slice(t * TT, (t + 1) * TT)
                    tsl128 = slice(t * TT, t * TT + 128)
                    if t < 2:
                        acc = vacc01[t]
                    else:
                        acc = vps.tile([TT, DIN], f32, name="vacc", tag="v",
                                       bufs=2)
                        nc.tensor.matmul(acc[:], ones1[0:1, :], bvr[0:1, :],
                                         start=True, stop=False,
                                         skip_group_check=True)
                    for k in range(4):
                        nc.tensor.matmul(acc[:], xt[k][:, tsl], wv[k][:],
                                         start=False, stop=(k == 3),
                                         skip_group_check=True)
                    vdat = vt[t].rearrange("p (h c) -> p h c", c=D + 1)[:, :, :D]
                    nc.vector.tensor_scalar_max(
                        vdat, acc.rearrange("p (h c) -> p h c", c=D), 0.0)
                    for hg in range(2):
                        heads = [hg * 4 + i for i in range(4)]
                        st2 = aps.tile([128, 1024], f32, name="st2", tag="s",
                                       bufs=2)
                        nc.tensor.matmul(st2[:, 0:2 * TT], ma[0:5, :],
                                         mb2[0:5, :], start=True, stop=False,
                                         skip_group_check=True)
                        nc.tensor.matmul(st2[:, 512:512 + 2 * TT],
                                         ma[64:69, :], mb2[64:69, :],
                                         start=True, stop=False,
                                         skip_group_check=True)
                        for i, h in enumerate(heads):
                            ft, po = h // 2, (h % 2) * 64
                            col = (h % 2) * 512 + (i // 2) * TT
                            nc.tensor.matmul(
                                st2[:, col:col + TT],
                                kt_[ft][po:po + 64, tsl128],
                                qt[ft][po:po + 64, tsl],
                                start=False, stop=(i >= 2),
                                skip_group_check=True)
                        # et2 padded to 432 cols so 128-wide PV stationary
                        # slices stay in bounds
                        et2 = sp.tile([TT, 4 * TT + 32], io_dt(pv_dt),
                                      name="et2", tag="et", bufs=4)
                        nc.scalar.activation(
                            et2[:, 0:4 * TT].rearrange("p (b c) -> p b c",
                                                       c=2 * TT),
                            st2[0:TT, :].rearrange("p (b c) -> p b c",
                                                   c=512)[:, :, 0:2 * TT],
                            AF.Exp)

                        # all 4 PVs of this unit share one PSUM bank (all
                        # their matmuls use rows 0-99 -> serialized, safe)
                        pv4 = pps2.tile([128, 4 * (D + 1)], f32, name="pv4",
                                        tag="pv", bufs=2)
                        for i, h in enumerate(heads):
                            col = (h % 2) * 2 * TT + (i // 2) * TT
                            nc.tensor.matmul(pv4[:, i * (D + 1):
                                                 (i + 1) * (D + 1)],
                                             et2[:, col:col + 128],
                                             vt[t][:, h * (D + 1):
                                                   (h + 1) * (D + 1)],
                                             start=True, stop=True,
                                             skip_group_check=True)
                        pv4v = pv4[0:TT, :].rearrange("p (h c) -> p h c",
                                                      c=D + 1)
                        rc4 = sp.tile([TT, 4], f32, name="rc4", tag="rc",
                                      bufs=8)
                        nc.vector.reciprocal(rc4[:], pv4v[:, :, D:D + 1])
                        nc.vector.tensor_mul(
                            ot[t][:, hg * 256:(hg + 1) * 256].rearrange(
                                "p (h c) -> p h c", c=D),
                            pv4v[:, :, 0:D],
                            rc4.unsqueeze(2).broadcast_to([TT, 4, D]))
                        if t == NT - 1:
                            # last tile: per-unit out DMA trims the tail
                            nc.sync.dma_start(
                                out=out_d[tsl, hg * 256:(hg + 1) * 256],
                                in_=ot[t][:, hg * 256:(hg + 1) * 256])
                    if t != NT - 1:
                        nc.sync.dma_start(out=out_d[tsl, :], in_=ot[t][:])

    nc.compile()
    return nc


def _prep_inputs(x, Wq, bq, Wk, bk, Wv, bv, proj_dt):
    import ml_dtypes

    x = np.asarray(x, np.float32)
    Wq = np.asarray(Wq, np.float32)
    bq = np.asarray(bq, np.float32)
    Wk = np.asarray(Wk, np.float32)
    bk = np.asarray(bk, np.float32)
    Wv = np.asarray(Wv, np.float32)
    bv = np.asarray(bv, np.float32)

    scale = 1.0 / np.sqrt(np.float32(D))  # 1/8, exact
    wq_s = (Wq * scale).astype(np.float32)
    bq_s = (bq * scale).astype(np.float32)

    io_np = ml_dtypes.bfloat16 if proj_dt == "bf16" else np.float32
    xT = np.ascontiguousarray(x.transpose(0, 2, 1))  # [B, DIN, N]

    bqc = np.ascontiguousarray(bq_s.reshape(4, 128).T)
    bkc = np.ascontiguousarray(bk.reshape(4, 128).T)
    bvr = bv[None, :].astype(ml_dtypes.bfloat16)

    # rank-5 factors of the additive frame mask over one 100-token tile,
    # replicated at partition rows 0-4 (bp0 banks) and 64-68 (bp64 banks).
    # mA is 128 cols wide: cols 100-127 (junk k-token rows of the padded
    # score stationary) get only the -big ones row -> exp -> 0.
    mA = np.zeros((128, 128), ml_dtypes.bfloat16)
    mB = np.zeros((128, TT), ml_dtypes.bfloat16)
    big = ml_dtypes.bfloat16(9e15)
    for base in (0, 64):
        mA[base, :] = 1
        mB[base, :] = -big
        for f in range(4):
            mA[base + 1 + f, f * JN:(f + 1) * JN] = 1
            mB[base + 1 + f, f * JN:(f + 1) * JN] = big
    mB2 = np.ascontiguousarray(np.tile(mB, (1, 2)))

    in_maps = []
    for c in range(NCORES):
        b, fb = c // 4, c % 4
        in_maps.append({
            "xT": np.ascontiguousarray(
                xT[b, :, fb * TOK:(fb + 1) * TOK]).astype(io_np),
            "wq": wq_s.astype(io_np),
            "wk": Wk.astype(io_np),
            "wv": Wv.astype(io_np),
            "bqc": bqc, "bkc": bkc, "bvr": bvr,
            "mA": mA, "mB2": mB2,
        })
    return in_maps


def kernel(x, Wq, bq, Wk, bk, Wv, bv, att_heads=H, latent_dim=D,
           time_len=TL, joint_num=JN, **_):
    from concourse.bass_utils import run_bass_kernel_spmd

    cfg = tuple(sorted(CONFIG.items()))
    if cfg not in _CACHE:
        _CACHE[cfg] = _build(CONFIG)
    nc = _CACHE[cfg]

    in_maps = _prep_inputs(x, Wq, bq, Wk, bk, Wv, bv, CONFIG["proj"])
    res = run_bass_kernel_spmd(nc, in_maps, core_ids=list(range(NCORES)))
    global LAST_RESULT
    LAST_RESULT = res

    out = np.empty((B, N, DIN), np.float32)
    for c in range(NCORES):
        b, fb = c // 4, c % 4
        out[b, fb * TOK:(fb + 1) * TOK, :] = res.results[c]["out"]
    return out
